# revision 25
# baseline (speedup 1.0000x reference)
"""Tensor-parallel llama-style attention (prefill) on 8 TRN2 NeuronCores.

Sharding: tensor-parallel over heads. Core c holds q-heads [4c, 4c+4),
kv-head c, the matching rows of wq/wk/wv, and columns [512c, 512c+512) of
wo. Each core computes a full-size partial of the output projection;
partials are summed on the host (the "all-reduce after wo").

Device-side layout tricks:
  - All activations are kept transposed (feature dim on partitions):
    xT [DIM, TOK], Q^T/K^T [128, S] per head, V in token-major chunks.
    The host pre-transposes x and the weight slices so every DMA is
    contiguous.
  - RoPE: the head dim basis is permuted on the host (even components
    first, odd second), which turns the interleaved rotation into a
    half-swap + elementwise mul/add with [128, S] cos/sin tables.
  - Softmax: no max-subtraction (scores*scale is O(10) here, exp is
    safe in fp32); masked-out blocks are exactly 0 after exp because
    exp(-1e9 * scale') underflows. Row sums via a ones-vector matmul on
    the tensor engine (partition reductions are impossible on DVE), and
    the 1/sum is broadcast back across partitions with a K=1 matmul.
  - All matmuls run in float32r (fp32 data, ~1e-4 rel err, 4x faster
    than fp32 on the PE when the moving free dim is >= 256).
"""

import math
import os
import sys

sys.path.insert(0, "/opt/trn_rl_repo")

import numpy as np

import concourse.bacc as bacc
import concourse.tile as tile
import concourse.mybir as mybir
from concourse import masks
from concourse.bass_utils import run_bass_kernel_spmd

B, S, DIM = 2, 2048, 4096
TOK = B * S
NH, NKV, HD = 32, 8, 128
NCORES = 8
HQ = NH // NCORES            # 4 query heads per core
SCALE = 1.0 / math.sqrt(HD)
F32 = mybir.dt.float32
F32R = mybir.dt.float32r
EXP = mybir.ActivationFunctionType.Exp

QB = 4          # q-blocks per batch (512 queries each)
QW = S // QB    # 512
KT = S // 128   # 16 k-tiles per batch


def _build(causal: bool, sps_bufs=3, pt_bufs=3, wps_bufs=3, p1ps_bufs=4,
           skip_norm=False, skip_exp=False, skip_mask=False,
           no_p1=False, no_a=False, no_w=False):
    nc = bacc.Bacc("TRN2", target_bir_lowering=False, debug=False)

    xT_d = nc.dram_tensor("xT", [DIM, TOK], F32R, kind="ExternalInput")
    w_d = nc.dram_tensor("wqkvT", [DIM, (HQ + 2) * HD], F32R, kind="ExternalInput")
    wo_d = nc.dram_tensor("woT", [HQ * HD, DIM], F32R, kind="ExternalInput")
    cos_d = nc.dram_tensor("cosT", [HD, S], F32, kind="ExternalInput")
    sin_d = nc.dram_tensor("sinTs", [HD, S], F32, kind="ExternalInput")
    nmask = 4 if causal else KT
    mask_d = nc.dram_tensor("maskTd", [QB, nmask, 128, QW], F32, kind="ExternalInput")
    out_d = nc.dram_tensor("out_part", [TOK, DIM], F32, kind="ExternalOutput")

    xT = xT_d.ap().rearrange("(kt p) t -> p kt t", p=128)      # [128, 32, TOK]
    w_ap = w_d.ap().rearrange("(kt p) j -> p kt j", p=128)     # [128, 32, 768]
    wo_ap = wo_d.ap().rearrange("(dt p) m -> p dt m", p=128)   # [128, 4, DIM]

    with tile.TileContext(nc) as tc:
        with (
            tc.tile_pool(name="const", bufs=1) as const_pool,
            tc.tile_pool(name="batch", bufs=1) as batch_pool,
            tc.tile_pool(name="dram", bufs=2, space="DRAM") as dram_pool,
        ):
            wqkv = const_pool.tile([128, 32, (HQ + 2) * HD], F32R)
            nc.sync.dma_start(wqkv[:], w_ap)
            cosT = const_pool.tile([HD, S], F32)
            sinTs = const_pool.tile([HD, S], F32)
            nc.sync.dma_start(cosT[:], cos_d.ap())
            nc.sync.dma_start(sinTs[:], sin_d.ap())
            ident = const_pool.tile([128, 128], F32)
            masks.make_identity(nc, ident[:])
            ones_f = const_pool.tile([128, 128], F32)
            nc.vector.memset(ones_f[:], 1.0)
            ones_col = const_pool.tile([128, 1], F32R)
            nc.vector.tensor_copy(ones_col[:], ones_f[:, 0:1])

            # per-batch K^T and V stay resident; Q^T and attn^T spill to DRAM
            kT_s = batch_pool.tile([128, S], F32R)
            v_s = batch_pool.tile([128, KT, HD], F32R)

            for b in range(B):
                qT_d = dram_pool.tile([HQ, HD, S], F32R)
                attnT_d = dram_pool.tile([HQ, HD, S], F32R)

                # ---------------- P1: QKV projections + RoPE ----------------
                with (
                    tc.tile_pool(name="xt", bufs=2) as xt_pool,
                    tc.tile_pool(name="rope", bufs=3) as rope_pool,
                    tc.tile_pool(name="vtmp", bufs=2) as vtmp_pool,
                    tc.tile_pool(name="p1ps", bufs=p1ps_bufs, space="PSUM") as p1ps,
                    tc.tile_pool(name="trps", bufs=2, space="PSUM") as trps,
                ):
                    for tb in range(0 if no_p1 else 8):   # 256-token chunks
                        c0 = b * S + tb * 256
                        sl = slice(tb * 256, tb * 256 + 256)   # seq positions
                        xt_c = xt_pool.tile([128, 32, 256], F32R)
                        nc.sync.dma_start(xt_c[:], xT[:, :, c0:c0 + 256])
                        for j in range(HQ + 2):
                            ps = p1ps.tile([128, 256], F32)
                            for k in range(32):
                                nc.tensor.matmul(
                                    ps[:], wqkv[:, k, j * HD:(j + 1) * HD],
                                    xt_c[:, k, :],
                                    start=(k == 0), stop=(k == 31))
                            if j < HQ + 1:
                                # RoPE: out = z*cosT + swap64(z)*sinTs
                                tmp = rope_pool.tile([128, 256], F32, tag="tmp")
                                nc.vector.tensor_mul(
                                    tmp[0:64, :], ps[64:128, :], sinTs[0:64, sl])
                                nc.vector.tensor_mul(
                                    tmp[64:128, :], ps[0:64, :], sinTs[64:128, sl])
                                t2 = rope_pool.tile([128, 256], F32, tag="t2")
                                nc.vector.tensor_mul(t2[:], ps[:], cosT[:, sl])
                                if j < HQ:
                                    rT = rope_pool.tile([128, 256], F32R, tag="rT")
                                    nc.vector.tensor_add(rT[:], t2[:], tmp[:])
                                    nc.sync.dma_start(
                                        qT_d[j, :, sl], rT[:])
                                else:
                                    nc.vector.tensor_add(
                                        kT_s[:, sl], t2[:], tmp[:])
                            else:
                                # V: copy out of PSUM, transpose to token-major
                                v_sb = vtmp_pool.tile([128, 256], F32)
                                nc.vector.tensor_copy(v_sb[:], ps[:])
                                for h2 in range(2):
                                    tp = trps.tile([128, 128], F32)
                                    nc.tensor.transpose(
                                        tp[:], v_sb[:, h2 * 128:(h2 + 1) * 128],
                                        ident[:])
                                    nc.vector.tensor_copy(
                                        v_s[:, tb * 2 + h2, :], tp[:])

                # ---------------- A: attention ----------------
                with (
                    tc.tile_pool(name="mask", bufs=2 if causal else 1) as mask_pool,
                    tc.tile_pool(name="qh", bufs=2) as q_pool,
                    tc.tile_pool(name="pT", bufs=pt_bufs) as p_pool,
                    tc.tile_pool(name="aT", bufs=2) as a_pool,
                    tc.tile_pool(name="rcp", bufs=2) as r_pool,
                    tc.tile_pool(name="sps", bufs=sps_bufs, space="PSUM") as sps,
                    tc.tile_pool(name="sums", bufs=2, space="PSUM") as sums_ps,
                    tc.tile_pool(name="ops", bufs=2, space="PSUM") as o_ps_pool,
                ):
                    for qb in range(0 if no_a else QB):
                        m_s = mask_pool.tile([128, nmask, QW], F32)
                        nc.sync.dma_start(
                            m_s[:],
                            mask_d.ap()[qb].rearrange("kt p q -> p kt q"))
                        nkt = 4 * (qb + 1) if causal else KT
                        kt0 = 4 * qb if causal else 0   # first masked k-tile
                        for h in range(HQ):
                            qh = q_pool.tile([128, QW], F32R)
                            nc.sync.dma_start(
                                qh[:], qT_d[h, :, qb * QW:(qb + 1) * QW])
                            sum_ps = sums_ps.tile([1, QW], F32)
                            o_ps = o_ps_pool.tile([128, QW], F32)
                            for kt in range(nkt):
                                s_ps = sps.tile([128, QW], F32, tag="s_ps")
                                nc.tensor.matmul(
                                    s_ps[:], kT_s[:, kt * 128:(kt + 1) * 128],
                                    qh[:], start=True, stop=True)
                                if kt >= kt0 and not skip_mask:
                                    nc.vector.tensor_add(
                                        s_ps[:], s_ps[:], m_s[:, kt - kt0, :])
                                pT = p_pool.tile([128, QW], F32R)
                                if skip_exp:
                                    nc.vector.tensor_copy(pT[:], s_ps[:])
                                else:
                                    nc.scalar.activation(
                                        pT[:], s_ps[:], EXP,
                                        bias=0.0, scale=SCALE)
                                if not skip_norm:
                                    nc.tensor.matmul(
                                        sum_ps[:], ones_col[:], pT[:],
                                        start=(kt == 0), stop=(kt == nkt - 1))
                                nc.tensor.matmul(
                                    o_ps[:], v_s[:, kt, :], pT[:],
                                    start=(kt == 0), stop=(kt == nkt - 1))
                            aT = a_pool.tile([128, QW], F32R)
                            if skip_norm:
                                nc.vector.tensor_copy(aT[:], o_ps[:])
                            else:
                                recip = r_pool.tile([1, QW], F32)
                                nc.vector.reciprocal(recip[:], sum_ps[:])
                                bc_sb = r_pool.tile([128, QW], F32, tag="bc")
                                nc.gpsimd.partition_broadcast(
                                    bc_sb[:], recip[:])
                                nc.vector.tensor_mul(
                                    aT[:], o_ps[:], bc_sb[:])
                            nc.sync.dma_start(
                                attnT_d[h, :, qb * QW:(qb + 1) * QW], aT[:])

                # ---------------- W: output projection partial ----------------
                with (
                    tc.tile_pool(name="atth", bufs=1) as att_pool,
                    tc.tile_pool(name="wo", bufs=2) as wo_pool,
                    tc.tile_pool(name="osb", bufs=3) as osb_pool,
                    tc.tile_pool(name="wps", bufs=wps_bufs, space="PSUM") as wps,
                ):
                    att_h = att_pool.tile([128, HQ, S], F32R)
                    for d4 in range(0 if no_w else HQ):
                        nc.sync.dma_start(att_h[:, d4, :], attnT_d[d4])
                    for mb in range(0 if no_w else 8):   # 512-wide output cols
                        wo_t = wo_pool.tile([128, HQ, 512], F32R)
                        nc.sync.dma_start(
                            wo_t[:], wo_ap[:, :, mb * 512:(mb + 1) * 512])
                        for tt in range(16):     # 128-token tiles
                            ps_w = wps.tile([128, 512], F32)
                            for d4 in range(HQ):
                                nc.tensor.matmul(
                                    ps_w[:],
                                    att_h[:, d4, tt * 128:(tt + 1) * 128],
                                    wo_t[:, d4, :],
                                    start=(d4 == 0), stop=(d4 == HQ - 1))
                            o_sb = osb_pool.tile([128, 512], F32)
                            nc.vector.tensor_copy(o_sb[:], ps_w[:])
                            row0 = b * S + tt * 128
                            nc.sync.dma_start(
                                out_d.ap()[row0:row0 + 128,
                                           mb * 512:(mb + 1) * 512],
                                o_sb[:])

    nc.compile()
    return nc


_CACHE = {}
LAST_EXEC_NS = None


def _get_nc(causal: bool):
    if causal not in _CACHE:
        _CACHE[causal] = _build(causal)
    return _CACHE[causal]


def _host_prep(x, wq, wk, wv, wo, freqs_cos, freqs_sin, mask):
    perm = np.concatenate([np.arange(0, HD, 2), np.arange(1, HD, 2)])
    wq_p = wq.reshape(NH, HD, DIM)[:, perm, :].reshape(NH * HD, DIM)
    wk_p = wk.reshape(NKV, HD, DIM)[:, perm, :].reshape(NKV * HD, DIM)

    xT = np.ascontiguousarray(x.reshape(TOK, DIM).T)

    cos = freqs_cos.T                     # [64, S]
    sin = freqs_sin.T
    cosT = np.ascontiguousarray(np.concatenate([cos, cos], 0))       # [128, S]
    sinTs = np.ascontiguousarray(np.concatenate([-sin, sin], 0))

    ref_mask = np.triu(np.full((S, S), -1e9, dtype=np.float32), k=1)
    causal = np.array_equal(mask, ref_mask)

    maskT = np.ascontiguousarray(mask.T) / np.float32(SCALE)   # [k, q]
    nmask = 4 if causal else KT
    maskTd = np.empty((QB, nmask, 128, QW), dtype=np.float32)
    for qb in range(QB):
        for j in range(nmask):
            kt = (4 * qb + j) if causal else j
            maskTd[qb, j] = maskT[kt * 128:(kt + 1) * 128,
                                  qb * QW:(qb + 1) * QW]

    in_maps = []
    for c in range(NCORES):
        wqT = wq_p[c * HQ * HD:(c + 1) * HQ * HD, :].T          # [DIM, 512]
        wkT = wk_p[c * HD:(c + 1) * HD, :].T                    # [DIM, 128]
        wvT = wv[c * HD:(c + 1) * HD, :].T                      # [DIM, 128]
        wqkvT = np.ascontiguousarray(np.concatenate([wqT, wkT, wvT], 1))
        woT = np.ascontiguousarray(wo[:, c * HQ * HD:(c + 1) * HQ * HD].T)
        in_maps.append({
            "xT": xT, "wqkvT": wqkvT, "woT": woT,
            "cosT": cosT, "sinTs": sinTs, "maskTd": maskTd,
        })
    return causal, in_maps


def kernel(x, wq, wk, wv, wo, freqs_cos, freqs_sin, mask, start_pos):
    global LAST_EXEC_NS
    x = np.asarray(x, dtype=np.float32)
    causal, in_maps = _host_prep(
        np.asarray(x, np.float32), np.asarray(wq, np.float32),
        np.asarray(wk, np.float32), np.asarray(wv, np.float32),
        np.asarray(wo, np.float32), np.asarray(freqs_cos, np.float32),
        np.asarray(freqs_sin, np.float32), np.asarray(mask, np.float32))

    nc = _get_nc(causal)
    res = run_bass_kernel_spmd(nc, in_maps, core_ids=list(range(NCORES)))
    LAST_EXEC_NS = res.exec_time_ns

    acc = res.results[0]["out_part"].astype(np.float64)
    for c in range(1, NCORES):
        acc += res.results[c]["out_part"]
    return acc.astype(np.float32).reshape(B, S, DIM)


if __name__ == "__main__":
    rng = np.random.default_rng(0)
    inputs = {
        "x": rng.standard_normal((B, S, DIM), dtype=np.float32),
        "wq": (rng.standard_normal((DIM, DIM), dtype=np.float32) * 0.02),
        "wk": (rng.standard_normal((NKV * HD, DIM), dtype=np.float32) * 0.02),
        "wv": (rng.standard_normal((NKV * HD, DIM), dtype=np.float32) * 0.02),
        "wo": (rng.standard_normal((DIM, DIM), dtype=np.float32) * 0.02),
        "freqs_cos": rng.random((S, HD // 2), dtype=np.float32),
        "freqs_sin": rng.random((S, HD // 2), dtype=np.float32),
        "mask": np.triu(np.full((S, S), -1e9, dtype=np.float32), k=1),
        "start_pos": 0,
    }
    out = kernel(**inputs)
    print("out", out.shape, out.dtype, float(np.abs(out).mean()))


# revision 26
# speedup vs baseline: 31.4181x; 31.4181x over previous
"""Tensor-parallel llama-style attention (prefill) on 8 TRN2 NeuronCores.

Sharding: tensor-parallel over heads. Core c holds q-heads [4c, 4c+4),
kv-head c, the matching rows of wq/wk/wv, and columns [512c, 512c+512) of
wo. Each core computes a full-size partial of the output projection;
partials are summed on the host (the "all-reduce after wo").

Device-side layout tricks:
  - All activations are kept transposed (feature dim on partitions):
    xT [DIM, TOK], Q^T/K^T [128, S] per head, V in token-major chunks.
    The host pre-transposes x and the weight slices so every DMA is
    contiguous.
  - RoPE: the head dim basis is permuted on the host (even components
    first, odd second), which turns the interleaved rotation into a
    half-swap + elementwise mul/add with [128, S] cos/sin tables.
  - Softmax: no max-subtraction (scores*scale is O(10) here, exp is
    safe in fp32); masked-out blocks are exactly 0 after exp because
    exp(-1e9 * scale') underflows. Row sums via a ones-vector matmul on
    the tensor engine (partition reductions are impossible on DVE), and
    the 1/sum is broadcast back across partitions with a K=1 matmul.
  - All matmuls run in float32r (fp32 data, ~1e-4 rel err, 4x faster
    than fp32 on the PE when the moving free dim is >= 256).
"""

import math
import os
import sys

sys.path.insert(0, "/opt/trn_rl_repo")

import numpy as np

import concourse.bacc as bacc
import concourse.tile as tile
import concourse.mybir as mybir
from concourse import masks
from concourse.bass_utils import run_bass_kernel_spmd

B, S, DIM = 2, 2048, 4096
TOK = B * S
NH, NKV, HD = 32, 8, 128
NCORES = 8
HQ = NH // NCORES            # 4 query heads per core
SCALE = 1.0 / math.sqrt(HD)
F32 = mybir.dt.float32
F32R = mybir.dt.float32r
EXP = mybir.ActivationFunctionType.Exp

QB = 4          # q-blocks per batch (512 queries each)
QW = S // QB    # 512
KT = S // 128   # 16 k-tiles per batch


def _build(causal: bool, sps_bufs=3, pt_bufs=3, wps_bufs=3, p1ps_bufs=4,
           skip_norm=False, skip_exp=False, skip_mask=False,
           no_p1=False, no_a=False, no_w=False):
    nc = bacc.Bacc("TRN2", target_bir_lowering=False, debug=False)

    xT_d = nc.dram_tensor("xT", [DIM, TOK], F32R, kind="ExternalInput")
    w_d = nc.dram_tensor("wqkvT", [DIM, (HQ + 2) * HD], F32R, kind="ExternalInput")
    wo_d = nc.dram_tensor("woT", [HQ * HD, DIM], F32R, kind="ExternalInput")
    cos_d = nc.dram_tensor("cosT", [HD, S], F32, kind="ExternalInput")
    sin_d = nc.dram_tensor("sinTs", [HD, S], F32, kind="ExternalInput")
    nmask = 4 if causal else KT
    mask_d = nc.dram_tensor("maskTd", [QB, nmask, 128, QW], F32, kind="ExternalInput")
    out_d = nc.dram_tensor("out_part", [TOK, DIM], F32, kind="ExternalOutput")

    xT = xT_d.ap().rearrange("(kt p) t -> p kt t", p=128)      # [128, 32, TOK]
    w_ap = w_d.ap().rearrange("(kt p) j -> p kt j", p=128)     # [128, 32, 768]
    wo_ap = wo_d.ap().rearrange("(dt p) m -> p dt m", p=128)   # [128, 4, DIM]

    with tile.TileContext(nc) as tc:
        with (
            tc.tile_pool(name="const", bufs=1) as const_pool,
            tc.tile_pool(name="batch", bufs=1) as batch_pool,
            tc.tile_pool(name="dram", bufs=2, space="DRAM") as dram_pool,
        ):
            wqkv = const_pool.tile([128, 32, (HQ + 2) * HD], F32R)
            nc.sync.dma_start(wqkv[:], w_ap)
            cosT = const_pool.tile([HD, S], F32)
            sinTs = const_pool.tile([HD, S], F32)
            nc.sync.dma_start(cosT[:], cos_d.ap())
            nc.sync.dma_start(sinTs[:], sin_d.ap())
            ident = const_pool.tile([128, 128], F32)
            masks.make_identity(nc, ident[:])
            ones_f = const_pool.tile([128, 128], F32)
            nc.vector.memset(ones_f[:], 1.0)
            ones_col = const_pool.tile([128, 1], F32R)
            nc.vector.tensor_copy(ones_col[:], ones_f[:, 0:1])

            # per-batch K^T and V stay resident; Q^T and attn^T spill to DRAM
            kT_s = batch_pool.tile([128, S], F32R)
            v_s = batch_pool.tile([128, KT, HD], F32R)

            for b in range(B):
                qT_d = dram_pool.tile([HQ, HD, S], F32R)
                attnT_d = dram_pool.tile([HQ, HD, S], F32R)

                # ---------------- P1: QKV projections + RoPE ----------------
                with (
                    tc.tile_pool(name="xt", bufs=2) as xt_pool,
                    tc.tile_pool(name="rope", bufs=3) as rope_pool,
                    tc.tile_pool(name="vtmp", bufs=2) as vtmp_pool,
                    tc.tile_pool(name="p1ps", bufs=p1ps_bufs, space="PSUM") as p1ps,
                    tc.tile_pool(name="trps", bufs=2, space="PSUM") as trps,
                ):
                    for tb in range(0 if no_p1 else 8):   # 256-token chunks
                        c0 = b * S + tb * 256
                        sl = slice(tb * 256, tb * 256 + 256)   # seq positions
                        xt_c = xt_pool.tile([128, 32, 256], F32R)
                        nc.sync.dma_start(xt_c[:], xT[:, :, c0:c0 + 256])
                        for j in range(HQ + 2):
                            ps = p1ps.tile([128, 256], F32)
                            for k in range(32):
                                nc.tensor.matmul(
                                    ps[:], wqkv[:, k, j * HD:(j + 1) * HD],
                                    xt_c[:, k, :],
                                    start=(k == 0), stop=(k == 31))
                            if j < HQ + 1:
                                # RoPE: out = z*cosT + swap64(z)*sinTs
                                tmp = rope_pool.tile([128, 256], F32, tag="tmp")
                                nc.vector.tensor_mul(
                                    tmp[0:64, :], ps[64:128, :], sinTs[0:64, sl])
                                nc.vector.tensor_mul(
                                    tmp[64:128, :], ps[0:64, :], sinTs[64:128, sl])
                                t2 = rope_pool.tile([128, 256], F32, tag="t2")
                                nc.vector.tensor_mul(t2[:], ps[:], cosT[:, sl])
                                if j < HQ:
                                    rT = rope_pool.tile([128, 256], F32R, tag="rT")
                                    nc.vector.tensor_add(rT[:], t2[:], tmp[:])
                                    nc.sync.dma_start(
                                        qT_d[j, :, sl], rT[:])
                                else:
                                    nc.vector.tensor_add(
                                        kT_s[:, sl], t2[:], tmp[:])
                            else:
                                # V: copy out of PSUM, transpose to token-major
                                v_sb = vtmp_pool.tile([128, 256], F32)
                                nc.vector.tensor_copy(v_sb[:], ps[:])
                                for h2 in range(2):
                                    tp = trps.tile([128, 128], F32)
                                    nc.tensor.transpose(
                                        tp[:], v_sb[:, h2 * 128:(h2 + 1) * 128],
                                        ident[:])
                                    nc.vector.tensor_copy(
                                        v_s[:, tb * 2 + h2, :], tp[:])

                # ---------------- A: attention ----------------
                with (
                    tc.tile_pool(name="mask", bufs=2 if causal else 1) as mask_pool,
                    tc.tile_pool(name="qh", bufs=2) as q_pool,
                    tc.tile_pool(name="pT", bufs=pt_bufs) as p_pool,
                    tc.tile_pool(name="aT", bufs=2) as a_pool,
                    tc.tile_pool(name="rcp", bufs=2) as r_pool,
                    tc.tile_pool(name="sps", bufs=sps_bufs, space="PSUM") as sps,
                    tc.tile_pool(name="sums", bufs=2, space="PSUM") as sums_ps,
                    tc.tile_pool(name="ops", bufs=2, space="PSUM") as o_ps_pool,
                ):
                    for qb in range(0 if no_a else QB):
                        m_s = mask_pool.tile([128, nmask, QW], F32)
                        nc.sync.dma_start(
                            m_s[:],
                            mask_d.ap()[qb].rearrange("kt p q -> p kt q"))
                        nkt = 4 * (qb + 1) if causal else KT
                        kt0 = 4 * qb if causal else 0   # first masked k-tile
                        for h in range(HQ):
                            qh = q_pool.tile([128, QW], F32R)
                            nc.sync.dma_start(
                                qh[:], qT_d[h, :, qb * QW:(qb + 1) * QW])
                            sum_ps = sums_ps.tile([1, QW], F32)
                            o_ps = o_ps_pool.tile([128, QW], F32)
                            for kt in range(nkt):
                                s_ps = sps.tile([128, QW], F32, tag="s_ps")
                                nc.tensor.matmul(
                                    s_ps[:], kT_s[:, kt * 128:(kt + 1) * 128],
                                    qh[:], start=True, stop=True)
                                if kt >= kt0 and not skip_mask:
                                    nc.vector.tensor_add(
                                        s_ps[:], s_ps[:], m_s[:, kt - kt0, :])
                                pT = p_pool.tile([128, QW], F32R)
                                if skip_exp:
                                    nc.vector.tensor_copy(pT[:], s_ps[:])
                                else:
                                    nc.scalar.activation(
                                        pT[:], s_ps[:], EXP,
                                        bias=0.0, scale=SCALE)
                                if not skip_norm:
                                    nc.tensor.matmul(
                                        sum_ps[:], ones_col[:], pT[:],
                                        start=(kt == 0), stop=(kt == nkt - 1))
                                nc.tensor.matmul(
                                    o_ps[:], v_s[:, kt, :], pT[:],
                                    start=(kt == 0), stop=(kt == nkt - 1))
                            aT = a_pool.tile([128, QW], F32R)
                            if skip_norm:
                                nc.vector.tensor_copy(aT[:], o_ps[:])
                            else:
                                recip = r_pool.tile([1, QW], F32)
                                nc.vector.reciprocal(recip[:], sum_ps[:])
                                bc_sb = r_pool.tile([128, QW], F32, tag="bc")
                                nc.gpsimd.partition_broadcast(
                                    bc_sb[:], recip[:])
                                nc.vector.tensor_mul(
                                    aT[:], o_ps[:], bc_sb[:])
                            nc.sync.dma_start(
                                attnT_d[h, :, qb * QW:(qb + 1) * QW], aT[:])

                # ---------------- W: output projection partial ----------------
                with (
                    tc.tile_pool(name="atth", bufs=1) as att_pool,
                    tc.tile_pool(name="wo", bufs=2) as wo_pool,
                    tc.tile_pool(name="osb", bufs=3) as osb_pool,
                    tc.tile_pool(name="wps", bufs=wps_bufs, space="PSUM") as wps,
                ):
                    att_h = att_pool.tile([128, HQ, S], F32R)
                    for d4 in range(0 if no_w else HQ):
                        nc.sync.dma_start(att_h[:, d4, :], attnT_d[d4])
                    out_v = out_d.ap().rearrange("(g p) m -> p g m", p=128)
                    for mb in range(0 if no_w else 8):   # 512-wide output cols
                        wo_t = wo_pool.tile([128, HQ, 512], F32R)
                        nc.sync.dma_start(
                            wo_t[:], wo_ap[:, :, mb * 512:(mb + 1) * 512])
                        for tg in range(4):      # groups of 4 token tiles
                            o_sb = osb_pool.tile([128, 4, 512], F32)
                            for ts in range(4):
                                tt = tg * 4 + ts
                                ps_w = wps.tile([128, 512], F32)
                                for d4 in range(HQ):
                                    nc.tensor.matmul(
                                        ps_w[:],
                                        att_h[:, d4, tt * 128:(tt + 1) * 128],
                                        wo_t[:, d4, :],
                                        start=(d4 == 0), stop=(d4 == HQ - 1))
                                nc.vector.tensor_copy(
                                    o_sb[:, ts, :], ps_w[:])
                            g0 = b * (S // 128) + tg * 4
                            nc.sync.dma_start(
                                out_v[:, g0:g0 + 4,
                                      mb * 512:(mb + 1) * 512],
                                o_sb[:])

    nc.compile()
    return nc


_CACHE = {}
LAST_EXEC_NS = None


def _get_nc(causal: bool):
    if causal not in _CACHE:
        _CACHE[causal] = _build(causal)
    return _CACHE[causal]


def _host_prep(x, wq, wk, wv, wo, freqs_cos, freqs_sin, mask):
    perm = np.concatenate([np.arange(0, HD, 2), np.arange(1, HD, 2)])
    wq_p = wq.reshape(NH, HD, DIM)[:, perm, :].reshape(NH * HD, DIM)
    wk_p = wk.reshape(NKV, HD, DIM)[:, perm, :].reshape(NKV * HD, DIM)

    xT = np.ascontiguousarray(x.reshape(TOK, DIM).T)

    cos = freqs_cos.T                     # [64, S]
    sin = freqs_sin.T
    cosT = np.ascontiguousarray(np.concatenate([cos, cos], 0))       # [128, S]
    sinTs = np.ascontiguousarray(np.concatenate([-sin, sin], 0))

    ref_mask = np.triu(np.full((S, S), -1e9, dtype=np.float32), k=1)
    causal = np.array_equal(mask, ref_mask)

    maskT = np.ascontiguousarray(mask.T) / np.float32(SCALE)   # [k, q]
    nmask = 4 if causal else KT
    maskTd = np.empty((QB, nmask, 128, QW), dtype=np.float32)
    for qb in range(QB):
        for j in range(nmask):
            kt = (4 * qb + j) if causal else j
            maskTd[qb, j] = maskT[kt * 128:(kt + 1) * 128,
                                  qb * QW:(qb + 1) * QW]

    in_maps = []
    for c in range(NCORES):
        wqT = wq_p[c * HQ * HD:(c + 1) * HQ * HD, :].T          # [DIM, 512]
        wkT = wk_p[c * HD:(c + 1) * HD, :].T                    # [DIM, 128]
        wvT = wv[c * HD:(c + 1) * HD, :].T                      # [DIM, 128]
        wqkvT = np.ascontiguousarray(np.concatenate([wqT, wkT, wvT], 1))
        woT = np.ascontiguousarray(wo[:, c * HQ * HD:(c + 1) * HQ * HD].T)
        in_maps.append({
            "xT": xT, "wqkvT": wqkvT, "woT": woT,
            "cosT": cosT, "sinTs": sinTs, "maskTd": maskTd,
        })
    return causal, in_maps


def kernel(x, wq, wk, wv, wo, freqs_cos, freqs_sin, mask, start_pos):
    global LAST_EXEC_NS
    x = np.asarray(x, dtype=np.float32)
    causal, in_maps = _host_prep(
        np.asarray(x, np.float32), np.asarray(wq, np.float32),
        np.asarray(wk, np.float32), np.asarray(wv, np.float32),
        np.asarray(wo, np.float32), np.asarray(freqs_cos, np.float32),
        np.asarray(freqs_sin, np.float32), np.asarray(mask, np.float32))

    nc = _get_nc(causal)
    res = run_bass_kernel_spmd(nc, in_maps, core_ids=list(range(NCORES)))
    LAST_EXEC_NS = res.exec_time_ns

    acc = res.results[0]["out_part"].astype(np.float64)
    for c in range(1, NCORES):
        acc += res.results[c]["out_part"]
    return acc.astype(np.float32).reshape(B, S, DIM)


if __name__ == "__main__":
    rng = np.random.default_rng(0)
    inputs = {
        "x": rng.standard_normal((B, S, DIM), dtype=np.float32),
        "wq": (rng.standard_normal((DIM, DIM), dtype=np.float32) * 0.02),
        "wk": (rng.standard_normal((NKV * HD, DIM), dtype=np.float32) * 0.02),
        "wv": (rng.standard_normal((NKV * HD, DIM), dtype=np.float32) * 0.02),
        "wo": (rng.standard_normal((DIM, DIM), dtype=np.float32) * 0.02),
        "freqs_cos": rng.random((S, HD // 2), dtype=np.float32),
        "freqs_sin": rng.random((S, HD // 2), dtype=np.float32),
        "mask": np.triu(np.full((S, S), -1e9, dtype=np.float32), k=1),
        "start_pos": 0,
    }
    out = kernel(**inputs)
    print("out", out.shape, out.dtype, float(np.abs(out).mean()))


# revision 29
# speedup vs baseline: 34.3973x; 1.0948x over previous
"""Tensor-parallel llama-style attention (prefill) on 8 TRN2 NeuronCores.

Sharding: tensor-parallel over heads. Core c holds q-heads [4c, 4c+4),
kv-head c, the matching rows of wq/wk/wv, and columns [512c, 512c+512) of
wo. Each core computes a full-size partial of the output projection;
partials are summed on the host (the "all-reduce after wo").

Device-side layout:
  - Activations are kept transposed (feature dim on partitions):
    xT [DIM, TOK], Q^T/K^T [128, S] per head, V in token-major chunks.
    The host pre-transposes x and the weight slices so every DMA is
    contiguous.
  - RoPE: the head-dim basis is permuted on the host (even components
    first, odd second), turning the interleaved rotation into a
    half-partition swap + elementwise mul/add against cos/sin tables.
    The swap reads the projection result directly from PSUM at a 64-
    partition offset (legal: only same-space operands must be aligned).
  - Softmax: no max-subtraction (scores*scale is O(10) here; exp is safe
    in fp32); masked-out blocks are exactly 0 after exp because
    exp(-1e9/scale*scale) underflows. Row sums via a ones-vector matmul
    on the tensor engine (DVE cannot reduce across partitions); 1/sum is
    broadcast back across partitions with gpsimd.partition_broadcast.
  - All matmuls run in float32r (fp32 bits, ~1.6e-4 rel err, 1 cyc/row
    when the moving free dim is >= 256 vs 4 for fp32). K=1 matmuls are
    broken in f32r on HW, hence the gpsimd broadcast.
  - P1 runs 6 PSUM accumulators in parallel (4 q-heads + K + V) over
    k-sliced xT streams, so each [128,128] weight LDWEIGHTS feeds an
    N=512 matmul.
"""

import math
import os
import sys

sys.path.insert(0, "/opt/trn_rl_repo")

import numpy as np

import concourse.bacc as bacc
import concourse.tile as tile
import concourse.mybir as mybir
from concourse import masks
from concourse.bass_utils import run_bass_kernel_spmd

B, S, DIM = 2, 2048, 4096
TOK = B * S
NH, NKV, HD = 32, 8, 128
NCORES = 8
HQ = NH // NCORES            # 4 query heads per core
SCALE = 1.0 / math.sqrt(HD)
F32 = mybir.dt.float32
F32R = mybir.dt.float32r
EXP = mybir.ActivationFunctionType.Exp

QB = 4          # q-blocks per batch (512 queries each)
QW = S // QB    # 512
KT = S // 128   # 16 k-tiles per batch
NJ = HQ + 2     # 6 projection output tiles: 4 Q heads, K, V


def _build(causal: bool):
    nc = bacc.Bacc("TRN2", target_bir_lowering=False, debug=False)

    xT_d = nc.dram_tensor("xT", [DIM, TOK], F32R, kind="ExternalInput")
    w_d = nc.dram_tensor("wqkvT", [DIM, NJ * HD], F32R, kind="ExternalInput")
    wo_d = nc.dram_tensor("woT", [HQ * HD, DIM], F32R, kind="ExternalInput")
    cos_d = nc.dram_tensor("cosT", [HD, S], F32, kind="ExternalInput")
    sin_d = nc.dram_tensor("sinTs", [HD, S], F32, kind="ExternalInput")
    nmask = 4 if causal else KT
    mask_d = nc.dram_tensor("maskTd", [QB, nmask, 128, QW], F32,
                            kind="ExternalInput")
    out_d = nc.dram_tensor("out_part", [TOK, DIM], F32, kind="ExternalOutput")

    xT = xT_d.ap().rearrange("(kt p) t -> p kt t", p=128)      # [128, 32, TOK]
    w_ap = w_d.ap().rearrange("(kt p) j -> p kt j", p=128)     # [128, 32, 768]
    wo_ap = wo_d.ap().rearrange("(dt p) m -> p dt m", p=128)   # [128, 4, DIM]
    out_v = out_d.ap().rearrange("(g p) m -> p g m", p=128)    # [128, 32, DIM]

    with tile.TileContext(nc) as tc:
        with (
            tc.tile_pool(name="const", bufs=1) as const_pool,
            tc.tile_pool(name="batch", bufs=1) as batch_pool,
            tc.tile_pool(name="dram", bufs=2, space="DRAM") as dram_pool,
        ):
            wqkv = const_pool.tile([128, 32, NJ * HD], F32R)
            for kc in range(4):     # chunked so P1 can start early
                nc.scalar.dma_start(wqkv[:, kc * 8:(kc + 1) * 8, :],
                                    w_ap[:, kc * 8:(kc + 1) * 8, :])
            ident = const_pool.tile([128, 128], F32)
            masks.make_identity(nc, ident[:])
            ones_f = const_pool.tile([128, 1], F32)
            nc.vector.memset(ones_f[:], 1.0)
            ones_col = const_pool.tile([128, 1], F32R)
            nc.vector.tensor_copy(ones_col[:], ones_f[:])

            # K^T, V, attn^T stay SBUF-resident per batch; Q^T spills to DRAM
            kT_s = batch_pool.tile([128, S], F32R)
            v_s = batch_pool.tile([128, KT, HD], F32R)
            att_h = batch_pool.tile([128, HQ, S], F32R)

            for b in range(B):
                qT_d = dram_pool.tile([HQ, HD, S], F32R)

                # ---------- P1: QKV projections + RoPE ----------
                with (
                    tc.tile_pool(name="xt", bufs=2) as xt_pool,
                    tc.tile_pool(name="cs", bufs=2) as cs_pool,
                    tc.tile_pool(name="rope", bufs=2) as rope_pool,
                    tc.tile_pool(name="vtmp", bufs=2) as vtmp_pool,
                    tc.tile_pool(name="p1ps", bufs=NJ, space="PSUM") as p1ps,
                    tc.tile_pool(name="trps", bufs=2, space="PSUM") as trps,
                ):
                    for tb in range(4):          # 512-token chunks
                        c0 = b * S + tb * 512
                        sl = slice(tb * 512, tb * 512 + 512)
                        cos_c = cs_pool.tile([HD, 512], F32, tag="cos")
                        sin_c = cs_pool.tile([HD, 512], F32, tag="sin")
                        nc.scalar.dma_start(cos_c[:], cos_d.ap()[:, sl])
                        nc.scalar.dma_start(sin_c[:], sin_d.ap()[:, sl])
                        pss = [p1ps.tile([128, 512], F32, tag="ps",
                                         name=f"ps{j}")
                               for j in range(NJ)]
                        for ks in range(4):      # k slices of 8 x-tiles
                            xt_c = xt_pool.tile([128, 8, 512], F32R, tag="xt")
                            nc.scalar.dma_start(
                                xt_c[:],
                                xT[:, ks * 8:(ks + 1) * 8, c0:c0 + 512])
                            for j in range(NJ):
                                for k in range(8):
                                    nc.tensor.matmul(
                                        pss[j][:],
                                        wqkv[:, ks * 8 + k,
                                             j * HD:(j + 1) * HD],
                                        xt_c[:, k, :],
                                        start=(ks == 0 and k == 0),
                                        stop=(ks == 3 and k == 7))
                        for j in range(NJ):
                            ps = pss[j]
                            if j < HQ + 1:
                                # RoPE: out = z*cos + swap64(z)*sin_signed
                                tmp = rope_pool.tile([128, 512], F32,
                                                     tag="tmp")
                                nc.vector.tensor_mul(
                                    tmp[0:64, :], ps[64:128, :],
                                    sin_c[0:64, :])
                                nc.vector.tensor_mul(
                                    tmp[64:128, :], ps[0:64, :],
                                    sin_c[64:128, :])
                                t2 = rope_pool.tile([128, 512], F32, tag="t2")
                                nc.vector.tensor_mul(t2[:], ps[:], cos_c[:])
                                if j < HQ:
                                    rT = rope_pool.tile([128, 512], F32R,
                                                        tag="rT")
                                    nc.vector.tensor_add(rT[:], t2[:], tmp[:])
                                    nc.sync.dma_start(qT_d[j, :, sl], rT[:])
                                else:
                                    nc.vector.tensor_add(
                                        kT_s[:, sl], t2[:], tmp[:])
                            else:
                                # V: copy from PSUM, transpose to token-major
                                v_sb = vtmp_pool.tile([128, 512], F32)
                                nc.vector.tensor_copy(v_sb[:], ps[:])
                                for h2 in range(4):
                                    tp = trps.tile([128, 128], F32)
                                    nc.tensor.transpose(
                                        tp[:],
                                        v_sb[:, h2 * 128:(h2 + 1) * 128],
                                        ident[:])
                                    nc.vector.tensor_copy(
                                        v_s[:, tb * 4 + h2, :], tp[:])

                # ---------- A: attention (writes att_h in SBUF) ----------
                with (
                    tc.tile_pool(name="mask",
                                 bufs=2 if causal else 1) as mask_pool,
                    tc.tile_pool(name="qh", bufs=3) as q_pool,
                    tc.tile_pool(name="pT", bufs=3) as p_pool,
                    tc.tile_pool(name="rcp", bufs=2) as r_pool,
                    tc.tile_pool(name="sps", bufs=3, space="PSUM") as sps,
                    tc.tile_pool(name="sums", bufs=2, space="PSUM") as sums_ps,
                    tc.tile_pool(name="ops", bufs=3, space="PSUM") as o_ps_pool,
                ):
                    for qb in range(QB):
                        m_s = mask_pool.tile([128, nmask, QW], F32)
                        nc.scalar.dma_start(
                            m_s[:],
                            mask_d.ap()[qb].rearrange("kt p q -> p kt q"))
                        nkt = 4 * (qb + 1) if causal else KT
                        kt0 = 4 * qb if causal else 0
                        for h in range(HQ):
                            qh = q_pool.tile([128, QW], F32R)
                            nc.sync.dma_start(
                                qh[:], qT_d[h, :, qb * QW:(qb + 1) * QW])
                            sum_ps = sums_ps.tile([1, QW], F32)
                            o_ps = o_ps_pool.tile([128, QW], F32)
                            for kt in range(nkt):
                                s_ps = sps.tile([128, QW], F32, tag="s_ps")
                                nc.tensor.matmul(
                                    s_ps[:], kT_s[:, kt * 128:(kt + 1) * 128],
                                    qh[:], start=True, stop=True)
                                if kt >= kt0:
                                    nc.vector.tensor_add(
                                        s_ps[:], s_ps[:], m_s[:, kt - kt0, :])
                                pT = p_pool.tile([128, QW], F32R)
                                nc.scalar.activation(
                                    pT[:], s_ps[:], EXP, bias=0.0,
                                    scale=SCALE)
                                nc.tensor.matmul(
                                    sum_ps[:], ones_col[:], pT[:],
                                    start=(kt == 0), stop=(kt == nkt - 1))
                                nc.tensor.matmul(
                                    o_ps[:], v_s[:, kt, :], pT[:],
                                    start=(kt == 0), stop=(kt == nkt - 1))
                            recip = r_pool.tile([1, QW], F32, tag="rcp")
                            nc.vector.reciprocal(recip[:], sum_ps[:])
                            bc_sb = r_pool.tile([128, QW], F32, tag="bc")
                            nc.gpsimd.partition_broadcast(bc_sb[:], recip[:])
                            nc.vector.tensor_mul(
                                att_h[:, h, qb * QW:(qb + 1) * QW],
                                o_ps[:], bc_sb[:])

                # ---------- W: output projection partial ----------
                with (
                    tc.tile_pool(name="wo", bufs=3) as wo_pool,
                    tc.tile_pool(name="osb", bufs=2) as osb_pool,
                    tc.tile_pool(name="wps", bufs=3, space="PSUM") as wps,
                ):
                    for mb in range(8):          # 512-wide output columns
                        wo_t = wo_pool.tile([128, HQ, 512], F32R)
                        nc.scalar.dma_start(
                            wo_t[:], wo_ap[:, :, mb * 512:(mb + 1) * 512])
                        for tg in range(4):      # groups of 4 token tiles
                            o_sb = osb_pool.tile([128, 4, 512], F32)
                            for ts in range(4):
                                tt = tg * 4 + ts
                                ps_w = wps.tile([128, 512], F32)
                                for d4 in range(HQ):
                                    nc.tensor.matmul(
                                        ps_w[:],
                                        att_h[:, d4, tt * 128:(tt + 1) * 128],
                                        wo_t[:, d4, :],
                                        start=(d4 == 0), stop=(d4 == HQ - 1))
                                nc.vector.tensor_copy(o_sb[:, ts, :], ps_w[:])
                            g0 = b * (S // 128) + tg * 4
                            nc.sync.dma_start(
                                out_v[:, g0:g0 + 4, mb * 512:(mb + 1) * 512],
                                o_sb[:])

    nc.compile()
    return nc


_CACHE = {}
LAST_EXEC_NS = None


def _get_nc(causal: bool):
    if causal not in _CACHE:
        _CACHE[causal] = _build(causal)
    return _CACHE[causal]


def _host_prep(x, wq, wk, wv, wo, freqs_cos, freqs_sin, mask):
    perm = np.concatenate([np.arange(0, HD, 2), np.arange(1, HD, 2)])
    wq_p = wq.reshape(NH, HD, DIM)[:, perm, :].reshape(NH * HD, DIM)
    wk_p = wk.reshape(NKV, HD, DIM)[:, perm, :].reshape(NKV * HD, DIM)

    xT = np.ascontiguousarray(x.reshape(TOK, DIM).T)

    cos = freqs_cos.T                     # [64, S]
    sin = freqs_sin.T
    cosT = np.ascontiguousarray(np.concatenate([cos, cos], 0))       # [128, S]
    sinTs = np.ascontiguousarray(np.concatenate([-sin, sin], 0))

    ref_mask = np.triu(np.full((S, S), -1e9, dtype=np.float32), k=1)
    causal = np.array_equal(mask, ref_mask)

    maskT = np.ascontiguousarray(mask.T) / np.float32(SCALE)   # [k, q]
    nmask = 4 if causal else KT
    maskTd = np.empty((QB, nmask, 128, QW), dtype=np.float32)
    for qb in range(QB):
        for j in range(nmask):
            kt = (4 * qb + j) if causal else j
            maskTd[qb, j] = maskT[kt * 128:(kt + 1) * 128,
                                  qb * QW:(qb + 1) * QW]

    in_maps = []
    for c in range(NCORES):
        wqT = wq_p[c * HQ * HD:(c + 1) * HQ * HD, :].T          # [DIM, 512]
        wkT = wk_p[c * HD:(c + 1) * HD, :].T                    # [DIM, 128]
        wvT = wv[c * HD:(c + 1) * HD, :].T                      # [DIM, 128]
        wqkvT = np.ascontiguousarray(np.concatenate([wqT, wkT, wvT], 1))
        woT = np.ascontiguousarray(wo[:, c * HQ * HD:(c + 1) * HQ * HD].T)
        in_maps.append({
            "xT": xT, "wqkvT": wqkvT, "woT": woT,
            "cosT": cosT, "sinTs": sinTs, "maskTd": maskTd,
        })
    return causal, in_maps


def kernel(x, wq, wk, wv, wo, freqs_cos, freqs_sin, mask, start_pos):
    global LAST_EXEC_NS
    causal, in_maps = _host_prep(
        np.asarray(x, np.float32), np.asarray(wq, np.float32),
        np.asarray(wk, np.float32), np.asarray(wv, np.float32),
        np.asarray(wo, np.float32), np.asarray(freqs_cos, np.float32),
        np.asarray(freqs_sin, np.float32), np.asarray(mask, np.float32))

    nc = _get_nc(causal)
    res = run_bass_kernel_spmd(nc, in_maps, core_ids=list(range(NCORES)))
    LAST_EXEC_NS = res.exec_time_ns

    acc = res.results[0]["out_part"].astype(np.float64)
    for c in range(1, NCORES):
        acc += res.results[c]["out_part"]
    return acc.astype(np.float32).reshape(B, S, DIM)


if __name__ == "__main__":
    rng = np.random.default_rng(0)
    inputs = {
        "x": rng.standard_normal((B, S, DIM), dtype=np.float32),
        "wq": (rng.standard_normal((DIM, DIM), dtype=np.float32) * 0.02),
        "wk": (rng.standard_normal((NKV * HD, DIM), dtype=np.float32) * 0.02),
        "wv": (rng.standard_normal((NKV * HD, DIM), dtype=np.float32) * 0.02),
        "wo": (rng.standard_normal((DIM, DIM), dtype=np.float32) * 0.02),
        "freqs_cos": rng.random((S, HD // 2), dtype=np.float32),
        "freqs_sin": rng.random((S, HD // 2), dtype=np.float32),
        "mask": np.triu(np.full((S, S), -1e9, dtype=np.float32), k=1),
        "start_pos": 0,
    }
    out = kernel(**inputs)
    print("out", out.shape, out.dtype, float(np.abs(out).mean()))


# revision 30
# speedup vs baseline: 35.4243x; 1.0299x over previous
"""Tensor-parallel llama-style attention (prefill) on 8 TRN2 NeuronCores.

Sharding: tensor-parallel over heads. Core c holds q-heads [4c, 4c+4),
kv-head c, the matching rows of wq/wk/wv, and columns [512c, 512c+512) of
wo. Each core computes a full-size partial of the output projection;
partials are summed on the host (the "all-reduce after wo").

Device-side layout:
  - Activations are kept transposed (feature dim on partitions):
    xT [DIM, TOK], Q^T/K^T [128, S] per head, V in token-major chunks.
    The host pre-transposes x and the weight slices so every DMA is
    contiguous.
  - RoPE: the head-dim basis is permuted on the host (even components
    first, odd second), turning the interleaved rotation into a
    half-partition swap + elementwise mul/add against cos/sin tables.
    The swap reads the projection result directly from PSUM at a 64-
    partition offset (legal: only same-space operands must be aligned).
  - Softmax: no max-subtraction (scores*scale is O(10) here; exp is safe
    in fp32); masked-out blocks are exactly 0 after exp because
    exp(-1e9/scale*scale) underflows. Row sums via a ones-vector matmul
    on the tensor engine (DVE cannot reduce across partitions); 1/sum is
    broadcast back across partitions with gpsimd.partition_broadcast.
  - All matmuls run in float32r (fp32 bits, ~1.6e-4 rel err, 1 cyc/row
    when the moving free dim is >= 256 vs 4 for fp32). K=1 matmuls are
    broken in f32r on HW, hence the gpsimd broadcast.
  - P1 runs 6 PSUM accumulators in parallel (4 q-heads + K + V) over
    k-sliced xT streams, so each [128,128] weight LDWEIGHTS feeds an
    N=512 matmul.
"""

import math
import os
import sys

sys.path.insert(0, "/opt/trn_rl_repo")

import numpy as np

import concourse.bacc as bacc
import concourse.tile as tile
import concourse.mybir as mybir
from concourse import masks
from concourse.bass_utils import run_bass_kernel_spmd

B, S, DIM = 2, 2048, 4096
TOK = B * S
NH, NKV, HD = 32, 8, 128
NCORES = 8
HQ = NH // NCORES            # 4 query heads per core
SCALE = 1.0 / math.sqrt(HD)
F32 = mybir.dt.float32
F32R = mybir.dt.float32r
EXP = mybir.ActivationFunctionType.Exp

QB = 4          # q-blocks per batch (512 queries each)
QW = S // QB    # 512
KT = S // 128   # 16 k-tiles per batch
NJ = HQ + 2     # 6 projection output tiles: 4 Q heads, K, V


def _build(causal: bool):
    nc = bacc.Bacc("TRN2", target_bir_lowering=False, debug=False)

    xT_d = nc.dram_tensor("xT", [DIM, TOK], F32R, kind="ExternalInput")
    w_d = nc.dram_tensor("wqkvT", [DIM, NJ * HD], F32R, kind="ExternalInput")
    wo_d = nc.dram_tensor("woT", [HQ * HD, DIM], F32R, kind="ExternalInput")
    cos_d = nc.dram_tensor("cosT", [HD, S], F32, kind="ExternalInput")
    sin_d = nc.dram_tensor("sinTs", [HD, S], F32, kind="ExternalInput")
    nmask = 4 if causal else KT
    mask_d = nc.dram_tensor("maskTd", [QB, nmask, 128, QW], F32,
                            kind="ExternalInput")
    out_d = nc.dram_tensor("out_part", [TOK, DIM], F32, kind="ExternalOutput")

    xT = xT_d.ap().rearrange("(kt p) t -> p kt t", p=128)      # [128, 32, TOK]
    w_ap = w_d.ap().rearrange("(kt p) j -> p kt j", p=128)     # [128, 32, 768]
    wo_ap = wo_d.ap().rearrange("(dt p) m -> p dt m", p=128)   # [128, 4, DIM]
    out_v = out_d.ap().rearrange("(g p) m -> p g m", p=128)    # [128, 32, DIM]

    with tile.TileContext(nc) as tc:
        with (
            tc.tile_pool(name="const", bufs=1) as const_pool,
            tc.tile_pool(name="batch", bufs=1) as batch_pool,
            tc.tile_pool(name="dram", bufs=2, space="DRAM") as dram_pool,
        ):
            wqkv = const_pool.tile([128, 32, NJ * HD], F32R)
            for kc in range(4):     # chunked so P1 can start early
                nc.scalar.dma_start(wqkv[:, kc * 8:(kc + 1) * 8, :],
                                    w_ap[:, kc * 8:(kc + 1) * 8, :])
            ident = const_pool.tile([128, 128], F32)
            masks.make_identity(nc, ident[:])
            ones_f = const_pool.tile([128, 1], F32)
            nc.vector.memset(ones_f[:], 1.0)
            ones_col = const_pool.tile([128, 1], F32R)
            nc.vector.tensor_copy(ones_col[:], ones_f[:])

            # K^T, V, attn^T stay SBUF-resident per batch; Q^T spills to DRAM
            kT_s = batch_pool.tile([128, S], F32R)
            v_s = batch_pool.tile([128, KT, HD], F32R)
            att_h = batch_pool.tile([128, HQ, S], F32R)

            for b in range(B):
                qT_d = dram_pool.tile([HQ, HD, S], F32R)

                # ---------- P1: QKV projections + RoPE ----------
                with (
                    tc.tile_pool(name="xt", bufs=2) as xt_pool,
                    tc.tile_pool(name="cs", bufs=2) as cs_pool,
                    tc.tile_pool(name="rope", bufs=2) as rope_pool,
                    tc.tile_pool(name="vtmp", bufs=2) as vtmp_pool,
                    tc.tile_pool(name="p1ps", bufs=NJ, space="PSUM") as p1ps,
                    tc.tile_pool(name="trps", bufs=2, space="PSUM") as trps,
                ):
                    for tb in range(4):          # 512-token chunks
                        c0 = b * S + tb * 512
                        sl = slice(tb * 512, tb * 512 + 512)
                        cos_c = cs_pool.tile([HD, 512], F32, tag="cos")
                        sin_c = cs_pool.tile([HD, 512], F32, tag="sin")
                        nc.sync.dma_start(cos_c[:], cos_d.ap()[:, sl])
                        nc.sync.dma_start(sin_c[:], sin_d.ap()[:, sl])
                        pss = [p1ps.tile([128, 512], F32, tag="ps",
                                         name=f"ps{j}")
                               for j in range(NJ)]
                        for ks in range(4):      # k slices of 8 x-tiles
                            xt_c = xt_pool.tile([128, 8, 512], F32R, tag="xt")
                            nc.sync.dma_start(
                                xt_c[:],
                                xT[:, ks * 8:(ks + 1) * 8, c0:c0 + 512])
                            for j in range(NJ):
                                for k in range(8):
                                    nc.tensor.matmul(
                                        pss[j][:],
                                        wqkv[:, ks * 8 + k,
                                             j * HD:(j + 1) * HD],
                                        xt_c[:, k, :],
                                        start=(ks == 0 and k == 0),
                                        stop=(ks == 3 and k == 7))
                        for j in range(NJ):
                            ps = pss[j]
                            if j < HQ + 1:
                                # RoPE: out = z*cos + swap64(z)*sin_signed
                                tmp = rope_pool.tile([128, 512], F32,
                                                     tag="tmp")
                                nc.vector.tensor_mul(
                                    tmp[0:64, :], ps[64:128, :],
                                    sin_c[0:64, :])
                                nc.vector.tensor_mul(
                                    tmp[64:128, :], ps[0:64, :],
                                    sin_c[64:128, :])
                                t2 = rope_pool.tile([128, 512], F32, tag="t2")
                                nc.vector.tensor_mul(t2[:], ps[:], cos_c[:])
                                if j < HQ:
                                    rT = rope_pool.tile([128, 512], F32R,
                                                        tag="rT")
                                    nc.vector.tensor_add(rT[:], t2[:], tmp[:])
                                    nc.sync.dma_start(qT_d[j, :, sl], rT[:])
                                else:
                                    nc.vector.tensor_add(
                                        kT_s[:, sl], t2[:], tmp[:])
                            else:
                                # V: copy from PSUM, transpose to token-major
                                v_sb = vtmp_pool.tile([128, 512], F32)
                                nc.vector.tensor_copy(v_sb[:], ps[:])
                                for h2 in range(4):
                                    tp = trps.tile([128, 128], F32)
                                    nc.tensor.transpose(
                                        tp[:],
                                        v_sb[:, h2 * 128:(h2 + 1) * 128],
                                        ident[:])
                                    nc.vector.tensor_copy(
                                        v_s[:, tb * 4 + h2, :], tp[:])

                # ---------- A: attention (writes att_h in SBUF) ----------
                with (
                    tc.tile_pool(name="mask",
                                 bufs=2 if causal else 1) as mask_pool,
                    tc.tile_pool(name="qh", bufs=3) as q_pool,
                    tc.tile_pool(name="pT", bufs=3) as p_pool,
                    tc.tile_pool(name="rcp", bufs=2) as r_pool,
                    tc.tile_pool(name="sps", bufs=3, space="PSUM") as sps,
                    tc.tile_pool(name="sums", bufs=2, space="PSUM") as sums_ps,
                    tc.tile_pool(name="ops", bufs=3, space="PSUM") as o_ps_pool,
                ):
                    for qb in range(QB):
                        m_s = mask_pool.tile([128, nmask, QW], F32)
                        nc.scalar.dma_start(
                            m_s[:],
                            mask_d.ap()[qb].rearrange("kt p q -> p kt q"))
                        nkt = 4 * (qb + 1) if causal else KT
                        kt0 = 4 * qb if causal else 0
                        for h in range(HQ):
                            qh = q_pool.tile([128, QW], F32R)
                            nc.sync.dma_start(
                                qh[:], qT_d[h, :, qb * QW:(qb + 1) * QW])
                            sum_ps = sums_ps.tile([1, QW], F32)
                            o_ps = o_ps_pool.tile([128, QW], F32)
                            for kt in range(nkt):
                                s_ps = sps.tile([128, QW], F32, tag="s_ps")
                                nc.tensor.matmul(
                                    s_ps[:], kT_s[:, kt * 128:(kt + 1) * 128],
                                    qh[:], start=True, stop=True)
                                if kt >= kt0:
                                    nc.vector.tensor_add(
                                        s_ps[:], s_ps[:], m_s[:, kt - kt0, :])
                                pT = p_pool.tile([128, QW], F32R)
                                nc.scalar.activation(
                                    pT[:], s_ps[:], EXP, bias=0.0,
                                    scale=SCALE)
                                nc.tensor.matmul(
                                    sum_ps[:], ones_col[:], pT[:],
                                    start=(kt == 0), stop=(kt == nkt - 1))
                                nc.tensor.matmul(
                                    o_ps[:], v_s[:, kt, :], pT[:],
                                    start=(kt == 0), stop=(kt == nkt - 1))
                            recip = r_pool.tile([1, QW], F32, tag="rcp")
                            nc.vector.reciprocal(recip[:], sum_ps[:])
                            bc_sb = r_pool.tile([128, QW], F32, tag="bc")
                            nc.gpsimd.partition_broadcast(bc_sb[:], recip[:])
                            nc.vector.tensor_mul(
                                att_h[:, h, qb * QW:(qb + 1) * QW],
                                o_ps[:], bc_sb[:])

                # ---------- W: output projection partial ----------
                with (
                    tc.tile_pool(name="wo", bufs=3) as wo_pool,
                    tc.tile_pool(name="osb", bufs=2) as osb_pool,
                    tc.tile_pool(name="wps", bufs=3, space="PSUM") as wps,
                ):
                    for mb in range(8):          # 512-wide output columns
                        wo_t = wo_pool.tile([128, HQ, 512], F32R)
                        nc.sync.dma_start(
                            wo_t[:], wo_ap[:, :, mb * 512:(mb + 1) * 512])
                        for tg in range(4):      # groups of 4 token tiles
                            o_sb = osb_pool.tile([128, 4, 512], F32)
                            for ts in range(4):
                                tt = tg * 4 + ts
                                ps_w = wps.tile([128, 512], F32)
                                for d4 in range(HQ):
                                    nc.tensor.matmul(
                                        ps_w[:],
                                        att_h[:, d4, tt * 128:(tt + 1) * 128],
                                        wo_t[:, d4, :],
                                        start=(d4 == 0), stop=(d4 == HQ - 1))
                                nc.vector.tensor_copy(o_sb[:, ts, :], ps_w[:])
                            g0 = b * (S // 128) + tg * 4
                            nc.sync.dma_start(
                                out_v[:, g0:g0 + 4, mb * 512:(mb + 1) * 512],
                                o_sb[:])

    nc.compile()
    return nc


_CACHE = {}
LAST_EXEC_NS = None


def _get_nc(causal: bool):
    if causal not in _CACHE:
        _CACHE[causal] = _build(causal)
    return _CACHE[causal]


def _host_prep(x, wq, wk, wv, wo, freqs_cos, freqs_sin, mask):
    perm = np.concatenate([np.arange(0, HD, 2), np.arange(1, HD, 2)])
    wq_p = wq.reshape(NH, HD, DIM)[:, perm, :].reshape(NH * HD, DIM)
    wk_p = wk.reshape(NKV, HD, DIM)[:, perm, :].reshape(NKV * HD, DIM)

    xT = np.ascontiguousarray(x.reshape(TOK, DIM).T)

    cos = freqs_cos.T                     # [64, S]
    sin = freqs_sin.T
    cosT = np.ascontiguousarray(np.concatenate([cos, cos], 0))       # [128, S]
    sinTs = np.ascontiguousarray(np.concatenate([-sin, sin], 0))

    ref_mask = np.triu(np.full((S, S), -1e9, dtype=np.float32), k=1)
    causal = np.array_equal(mask, ref_mask)

    maskT = np.ascontiguousarray(mask.T) / np.float32(SCALE)   # [k, q]
    nmask = 4 if causal else KT
    maskTd = np.empty((QB, nmask, 128, QW), dtype=np.float32)
    for qb in range(QB):
        for j in range(nmask):
            kt = (4 * qb + j) if causal else j
            maskTd[qb, j] = maskT[kt * 128:(kt + 1) * 128,
                                  qb * QW:(qb + 1) * QW]

    in_maps = []
    for c in range(NCORES):
        wqT = wq_p[c * HQ * HD:(c + 1) * HQ * HD, :].T          # [DIM, 512]
        wkT = wk_p[c * HD:(c + 1) * HD, :].T                    # [DIM, 128]
        wvT = wv[c * HD:(c + 1) * HD, :].T                      # [DIM, 128]
        wqkvT = np.ascontiguousarray(np.concatenate([wqT, wkT, wvT], 1))
        woT = np.ascontiguousarray(wo[:, c * HQ * HD:(c + 1) * HQ * HD].T)
        in_maps.append({
            "xT": xT, "wqkvT": wqkvT, "woT": woT,
            "cosT": cosT, "sinTs": sinTs, "maskTd": maskTd,
        })
    return causal, in_maps


def kernel(x, wq, wk, wv, wo, freqs_cos, freqs_sin, mask, start_pos):
    global LAST_EXEC_NS
    causal, in_maps = _host_prep(
        np.asarray(x, np.float32), np.asarray(wq, np.float32),
        np.asarray(wk, np.float32), np.asarray(wv, np.float32),
        np.asarray(wo, np.float32), np.asarray(freqs_cos, np.float32),
        np.asarray(freqs_sin, np.float32), np.asarray(mask, np.float32))

    nc = _get_nc(causal)
    res = run_bass_kernel_spmd(nc, in_maps, core_ids=list(range(NCORES)))
    LAST_EXEC_NS = res.exec_time_ns

    acc = res.results[0]["out_part"].astype(np.float64)
    for c in range(1, NCORES):
        acc += res.results[c]["out_part"]
    return acc.astype(np.float32).reshape(B, S, DIM)


if __name__ == "__main__":
    rng = np.random.default_rng(0)
    inputs = {
        "x": rng.standard_normal((B, S, DIM), dtype=np.float32),
        "wq": (rng.standard_normal((DIM, DIM), dtype=np.float32) * 0.02),
        "wk": (rng.standard_normal((NKV * HD, DIM), dtype=np.float32) * 0.02),
        "wv": (rng.standard_normal((NKV * HD, DIM), dtype=np.float32) * 0.02),
        "wo": (rng.standard_normal((DIM, DIM), dtype=np.float32) * 0.02),
        "freqs_cos": rng.random((S, HD // 2), dtype=np.float32),
        "freqs_sin": rng.random((S, HD // 2), dtype=np.float32),
        "mask": np.triu(np.full((S, S), -1e9, dtype=np.float32), k=1),
        "start_pos": 0,
    }
    out = kernel(**inputs)
    print("out", out.shape, out.dtype, float(np.abs(out).mean()))
